# revision 23
# baseline (speedup 1.0000x reference)
"""CorrPyramid kernel for Trainium2 (8 NeuronCores, Bass/Tile).

Computation (per batch sample, data-parallel over B=8 across 8 cores):
  3 pyramid levels; at each level, a 9-shift channel-mean correlation
  volume over C=256, with 2x2 avg-pooling of the feature maps between
  levels.

Per-core strategy:
  - Stream f1/f2 [256, H, W] from HBM exactly once, in h-chunks
    (~1-2 MB DMAs, double-buffered, split across both HWDGE rings).
  - Correlation via Gram matmuls on the PE: for each row h,
    lhsT = f1[:, h, :] (K=C on partitions, M=W), rhs = f2 row pair
    (N=256 so the float32r full-rate mode engages), accumulated over
    the two 128-channel halves in PSUM.  The useful [W, W] Gram block
    (f1 row h x f2 row h) is copied PSUM->SBUF contiguously (DVE,
    fp16 cast at level 0) and DMA'd to DRAM as G[w, h, w'].
  - The 9 correlation diagonals corr[d, h, w] = G[w, h, w+d-4] / C
    are extracted on the host with numpy fancy indexing: a skewed
    (per-partition-offset) access is not expressible by any on-chip
    engine AP, and per-partition DMAs cost ~0.5 us of DGE issue each.
    The full Gram rows ship instead (fp16 halves the level-0 bytes;
    fp16 adds <=2^-11 relative error, measured 3.9e-4 worst).
  - Pooling is a sum-pool (two strided tensor_adds; h-adds split
    f1->DVE / f2->GPSIMD, w-adds on DVE); the 1/4-per-level average
    factors and the 1/C mean fold into the host-side scale.  Pooling
    emission lags the stream loop by one chunk so the scheduler
    prioritizes the PSUM-recycling Gram copies.  Levels 1/2 run from
    SBUF-resident pooled slabs - no second HBM read; their Gram
    matmuls interleave into the stream loop as rows become available.

HW time is DMA-bound: 25.2 MB in + 4.0 MB Gram out per core
(~92.7 us/core on the instruction-cost-model timeline; DMA engines
87% occupied).
"""

import numpy as np

MAX_DISP = 4
ND = 2 * MAX_DISP + 1  # 9 shifts
B, C, H0, W0 = 8, 256, 96, 128
CH = C // 128  # channel halves (K tiles)

# (W, H, rows-per-rhs-group) per level; rhs N = Grp*W = 256 for float32r rate
LEVELS = [(128, 96, 2), (64, 48, 4), (32, 24, 8)]
# host-side output scales: 1/C, then extra 1/16 per level (sum-pool both operands)
SCALES = [1.0 / 256, 1.0 / 4096, 1.0 / 65536]

# h-rows per streamed chunk at level 0: small first chunk to start compute
# sooner, small last chunk to shorten the dependent pool->L1->L2 tail
CHUNKS = [8, 16, 16, 16, 16, 16, 8]

_cached = {}


def _build(chunks=None):
    import concourse.bass as bass
    import concourse.bacc as bacc
    import concourse.mybir as mybir
    from concourse import tile

    f32 = mybir.dt.float32
    f32r = mybir.dt.float32r

    nc = bacc.Bacc("TRN2", target_bir_lowering=False, num_devices=8)
    f1 = nc.dram_tensor("f1", [CH, 128, H0, W0], f32r, kind="ExternalInput")
    f2 = nc.dram_tensor("f2", [CH, 128, H0, W0], f32r, kind="ExternalInput")
    f16 = mybir.dt.float16
    gouts = [
        nc.dram_tensor(f"g{l}", [W, H * W], f16 if l == 0 else f32,
                       kind="ExternalOutput")
        for l, (W, H, _) in enumerate(LEVELS)
    ]

    W1, H1 = LEVELS[1][0], LEVELS[1][1]
    W2, H2 = LEVELS[2][0], LEVELS[2][1]

    chunk_list = list(CHUNKS if chunks is None else chunks)
    assert sum(chunk_list) == H0 and all(c % 8 == 0 for c in chunk_list)

    with tile.TileContext(nc) as tc:
        with (
            tc.tile_pool(name="stream", bufs=2) as stream,
            tc.tile_pool(name="gc", bufs=3) as gcp,
            tc.tile_pool(name="slab", bufs=1) as slab,
            tc.tile_pool(name="tmp", bufs=2) as tmp,
            tc.tile_pool(name="psum", bufs=8, space="PSUM") as psum,
        ):
            # level-1/2 pooled slabs, free = ch*(H*W) + h*W + w
            f1p = slab.tile([128, CH * H1 * W1], f32r)
            f2p = slab.tile([128, CH * H1 * W1], f32r)
            f1q = slab.tile([128, CH * H2 * W2], f32r)
            f2q = slab.tile([128, CH * H2 * W2], f32r)
            # level-1/2 Gram staging (natural layout G[w, h*W + w'])
            G1 = slab.tile([W1, H1 * W1], f32)
            G2 = slab.tile([W2, H2 * W2], f32)

            def sub_gram(l, g0, ngroups):
                """Matmuls+copies+store for L1/L2 rhs-groups starting at g0."""
                W, H, Grp = LEVELS[l]
                s1, s2 = (f1p, f2p) if l == 1 else (f1q, f2q)
                Gt = G1 if l == 1 else G2
                for g in range(g0, g0 + ngroups * Grp, Grp):
                    for t in range(Grp):
                        h = g + t
                        ps = psum.tile([128, 512], f32, tag="ps")
                        for ch in range(CH):
                            lhsT = s1[:, ch * H * W + h * W : ch * H * W + (h + 1) * W]
                            rhs = s2[:, ch * H * W + g * W : ch * H * W + (g + Grp) * W]
                            nc.tensor.matmul(
                                ps[:W, 0 : Grp * W], lhsT, rhs,
                                start=(ch == 0), stop=(ch == CH - 1),
                            )
                        nc.scalar.copy(
                            out=Gt[:, h * W : (h + 1) * W],
                            in_=ps[:W, t * W : (t + 1) * W],
                        )
                lo, hi = g0 * W, (g0 + ngroups * Grp) * W
                nc.sync.dma_start(out=gouts[l].ap()[:, lo:hi], in_=Gt[:, lo:hi])

            def emit_pool(h0, chunk, f1c, f2c):
                """Pooling + dependent L1/L2 gram work for one streamed chunk."""
                hc = chunk // 2
                qc = chunk // 4
                # ---- level0 -> level1 slab rows [h0/2, h0/2 + hc) ----
                t1s = []
                for idx, (t_sb, eng_h) in enumerate(((f1c, nc.vector), (f2c, nc.gpsimd))):
                    t1 = tmp.tile([128, CH * hc * W0], f32, tag=f"t1{idx}",
                                  name=f"t1{idx}")
                    a0 = bass.AP(t_sb.tensor, 0,
                                 [[t_sb.tensor.shape[1], 128],
                                  [chunk * W0, CH], [2 * W0, hc], [1, W0]])
                    a1 = bass.AP(t_sb.tensor, W0,
                                 [[t_sb.tensor.shape[1], 128],
                                  [chunk * W0, CH], [2 * W0, hc], [1, W0]])
                    d1 = bass.AP(t1.tensor, 0,
                                 [[t1.tensor.shape[1], 128],
                                  [hc * W0, CH], [W0, hc], [1, W0]])
                    eng_h.tensor_add(out=d1, in0=a0, in1=a1)
                    t1s.append(t1)
                for t1, t_slab in zip(t1s, (f1p, f2p)):
                    b0 = bass.AP(t1.tensor, 0,
                                 [[t1.tensor.shape[1], 128],
                                  [hc * W0, CH], [W0, hc], [2, W1]])
                    b1 = bass.AP(t1.tensor, 1,
                                 [[t1.tensor.shape[1], 128],
                                  [hc * W0, CH], [W0, hc], [2, W1]])
                    d2 = bass.AP(t_slab.tensor, (h0 // 2) * W1,
                                 [[t_slab.tensor.shape[1], 128],
                                  [H1 * W1, CH], [W1, hc], [1, W1]])
                    nc.vector.tensor_add(out=d2, in0=b0, in1=b1)

                # ---- level-1 Grams for rows just pooled ----
                sub_gram(1, h0 // 2, hc // LEVELS[1][2])

                # ---- level1 -> level2 slab rows [h0/4, h0/4 + qc) ----
                for t_slab, t_out in ((f1p, f1q), (f2p, f2q)):
                    eng = nc.vector if t_slab is f1p else nc.gpsimd
                    for ch in range(CH):
                        t2 = tmp.tile([128, qc * W1], f32, tag="t2", name="t2")
                        a0 = bass.AP(t_slab.tensor,
                                     ch * H1 * W1 + (h0 // 2) * W1,
                                     [[t_slab.tensor.shape[1], 128],
                                      [2 * W1, qc], [1, W1]])
                        a1 = bass.AP(t_slab.tensor,
                                     ch * H1 * W1 + (h0 // 2 + 1) * W1,
                                     [[t_slab.tensor.shape[1], 128],
                                      [2 * W1, qc], [1, W1]])
                        d1 = bass.AP(t2.tensor, 0,
                                     [[t2.tensor.shape[1], 128], [W1, qc], [1, W1]])
                        eng.tensor_add(out=d1, in0=a0, in1=a1)
                        b0 = bass.AP(t2.tensor, 0,
                                     [[t2.tensor.shape[1], 128], [W1, qc], [2, W2]])
                        b1 = bass.AP(t2.tensor, 1,
                                     [[t2.tensor.shape[1], 128], [W1, qc], [2, W2]])
                        d2 = bass.AP(t_out.tensor,
                                     ch * H2 * W2 + (h0 // 4) * W2,
                                     [[t_out.tensor.shape[1], 128],
                                      [W2, qc], [1, W2]])
                        nc.vector.tensor_add(out=d2, in0=b0, in1=b1)

                # ---- level-2 Grams whenever a full rhs-group (8 rows) ready ----
                Grp2 = LEVELS[2][2]
                prev_done = h0 // 4
                q_done = prev_done + qc
                for gidx in range(prev_done // Grp2, q_done // Grp2):
                    sub_gram(2, gidx * Grp2, 1)

            # ---------------- streamed pipeline ----------------
            pending_pool = None
            h0 = 0
            for ci, chunk in enumerate(chunk_list):
                f1c = stream.tile([128, CH * chunk * W0], f32r, tag="f1c", name="f1c")
                f2c = stream.tile([128, CH * chunk * W0], f32r, tag="f2c", name="f2c")
                for t_dram, t_sb, dma_eng in ((f1, f1c, nc.sync), (f2, f2c, nc.scalar)):
                    src = t_dram.ap()[:, :, h0 : h0 + chunk, :].transpose([1, 0, 2, 3])
                    dst = t_sb[:].rearrange("p (a b) -> p a b", a=CH)
                    dma_eng.dma_start(out=dst, in_=src)

                gc = gcp.tile([128, chunk * W0], f16, tag="gc", name="gc")
                # Gram matmuls: row pairs (ha, ha+1); one PSUM bank per pair
                for hp in range(chunk // 2):
                    ha = 2 * hp
                    ps = psum.tile([128, 512], f32, tag="ps", name="ps")
                    for t in range(2):
                        for ch in range(CH):
                            lhsT = f1c[:, ch * chunk * W0 + (ha + t) * W0:
                                       ch * chunk * W0 + (ha + t + 1) * W0]
                            rhs = f2c[:, ch * chunk * W0 + ha * W0:
                                      ch * chunk * W0 + (ha + 2) * W0]
                            nc.tensor.matmul(
                                ps[:, t * 256 : (t + 1) * 256], lhsT, rhs,
                                start=(ch == 0), stop=(ch == CH - 1),
                            )
                    # useful blocks: row ha -> ps[:,0:128]; row ha+1 -> ps[:,384:512]
                    # iteration (w, t, w'); contiguous dest gc[w, (ha+t)*128 + w']
                    src = bass.AP(
                        ps.tensor, 0, [[ps.tensor.shape[1], 128], [384, 2], [1, 128]]
                    )
                    dst = gc[:, ha * W0 : (ha + 2) * W0]
                    nc.vector.tensor_copy(out=dst, in_=src)
                nc.sync.dma_start(
                    out=gouts[0].ap()[:, h0 * W0 : (h0 + chunk) * W0], in_=gc[:]
                )

                # emit previous chunk's pooling AFTER this chunk's copies so
                # the scheduler prioritizes PSUM recycling
                if pending_pool is not None:
                    pending_pool()
                if ci == len(chunk_list) - 1:
                    # last chunk: emit its pooling chain immediately so the
                    # dependent L1/L2 tail starts as early as possible
                    emit_pool(h0, chunk, f1c, f2c)
                    pending_pool = None
                else:
                    pending_pool = (lambda h0=h0, chunk=chunk, f1c=f1c, f2c=f2c:
                                    emit_pool(h0, chunk, f1c, f2c))
                h0 += chunk
            if pending_pool is not None:
                pending_pool()

    nc.compile()
    return nc


def _get_nc():
    if "nc" not in _cached:
        _cached["nc"] = _build()
    return _cached["nc"]


def _extract(gfull, W, H, scale):
    """gfull [B, W, H*W] -> corr [B, 9, H, W]; corr[b,d,h,w] = g[b,w,h,w+d-4]*scale."""
    g = gfull.reshape(B, W, H, W)
    out = np.zeros((B, ND, H, W), np.float32)
    for d in range(ND):
        dp = d - MAX_DISP
        lo = max(0, -dp)
        hi = min(W, W - dp)
        wv = np.arange(lo, hi)
        sl = g[:, wv, :, wv + dp].astype(np.float32)  # [len(wv), B, H]
        out[:, d, :, lo:hi] = np.moveaxis(sl, 0, -1) * np.float32(scale)
    return out


def kernel(f1: np.ndarray, f2: np.ndarray):
    from concourse.bass_utils import run_bass_kernel_spmd

    nc = _get_nc()
    f1 = np.ascontiguousarray(np.asarray(f1, dtype=np.float32))
    f2 = np.ascontiguousarray(np.asarray(f2, dtype=np.float32))
    in_maps = [
        {
            "f1": f1[b].reshape(CH, 128, H0, W0),
            "f2": f2[b].reshape(CH, 128, H0, W0),
        }
        for b in range(B)
    ]
    res = run_bass_kernel_spmd(
        nc, in_maps, core_ids=list(range(B)), trace=_cached.get("trace", False)
    )
    _cached["last_result"] = res

    outs = []
    for l, (W, H, _) in enumerate(LEVELS):
        gfull = np.stack([res.results[b][f"g{l}"] for b in range(B)], axis=0)
        outs.append(_extract(gfull, W, H, SCALES[l]))
    return tuple(outs)


# revision 24
# speedup vs baseline: 1.0073x; 1.0073x over previous
"""CorrPyramid kernel for Trainium2 (8 NeuronCores, Bass/Tile).

Computation (per batch sample, data-parallel over B=8 across 8 cores):
  3 pyramid levels; at each level, a 9-shift channel-mean correlation
  volume over C=256, with 2x2 avg-pooling of the feature maps between
  levels.

Per-core strategy:
  - Stream f1/f2 [256, H, W] from HBM exactly once, in h-chunks
    (~1-2 MB DMAs, double-buffered, split across both HWDGE rings).
  - Correlation via Gram matmuls on the PE: for each row h,
    lhsT = f1[:, h, :] (K=C on partitions, M=W), rhs = f2 row pair
    (N=256 so the float32r full-rate mode engages), accumulated over
    the two 128-channel halves in PSUM.  The useful [W, W] Gram block
    (f1 row h x f2 row h) is copied PSUM->SBUF contiguously (DVE,
    fp16 cast at level 0) and DMA'd to DRAM as G[w, h, w'].
  - The 9 correlation diagonals corr[d, h, w] = G[w, h, w+d-4] / C
    are extracted on the host with numpy fancy indexing: a skewed
    (per-partition-offset) access is not expressible by any on-chip
    engine AP, and per-partition DMAs cost ~0.5 us of DGE issue each.
    The full Gram rows ship instead (fp16 halves the level-0 bytes;
    fp16 adds <=2^-11 relative error, measured 3.9e-4 worst).
  - Pooling is a sum-pool (two strided tensor_adds; h-adds split
    f1->DVE / f2->GPSIMD, w-adds on DVE); the 1/4-per-level average
    factors and the 1/C mean fold into the host-side scale.  Pooling
    emission lags the stream loop by one chunk so the scheduler
    prioritizes the PSUM-recycling Gram copies.  Levels 1/2 run from
    SBUF-resident pooled slabs - no second HBM read; their Gram
    matmuls interleave into the stream loop as rows become available.

HW time is DMA-bound: 25.2 MB in + 4.0 MB Gram out per core
(~92.7 us/core on the instruction-cost-model timeline; DMA engines
87% occupied).
"""

import numpy as np

MAX_DISP = 4
ND = 2 * MAX_DISP + 1  # 9 shifts
B, C, H0, W0 = 8, 256, 96, 128
CH = C // 128  # channel halves (K tiles)

# (W, H, rows-per-rhs-group) per level; rhs N = Grp*W = 256 for float32r rate
LEVELS = [(128, 96, 2), (64, 48, 4), (32, 24, 8)]
# host-side output scales: 1/C, then extra 1/16 per level (sum-pool both operands)
SCALES = [1.0 / 256, 1.0 / 4096, 1.0 / 65536]

# h-rows per streamed chunk at level 0: small first chunk to start compute
# sooner, small last chunk to shorten the dependent pool->L1->L2 tail
CHUNKS = [8, 16, 16, 16, 16, 16, 8]

_cached = {}


def _build(chunks=None):
    import concourse.bass as bass
    import concourse.bacc as bacc
    import concourse.mybir as mybir
    from concourse import tile

    f32 = mybir.dt.float32
    f32r = mybir.dt.float32r

    nc = bacc.Bacc("TRN2", target_bir_lowering=False, num_devices=8)
    f1 = nc.dram_tensor("f1", [CH, 128, H0, W0], f32r, kind="ExternalInput")
    f2 = nc.dram_tensor("f2", [CH, 128, H0, W0], f32r, kind="ExternalInput")
    f16 = mybir.dt.float16
    gouts = [
        nc.dram_tensor(f"g{l}", [W, H * W], f16, kind="ExternalOutput")
        for l, (W, H, _) in enumerate(LEVELS)
    ]

    W1, H1 = LEVELS[1][0], LEVELS[1][1]
    W2, H2 = LEVELS[2][0], LEVELS[2][1]

    chunk_list = list(CHUNKS if chunks is None else chunks)
    assert sum(chunk_list) == H0 and all(c % 8 == 0 for c in chunk_list)

    with tile.TileContext(nc) as tc:
        with (
            tc.tile_pool(name="stream", bufs=2) as stream,
            tc.tile_pool(name="gc", bufs=3) as gcp,
            tc.tile_pool(name="slab", bufs=1) as slab,
            tc.tile_pool(name="tmp", bufs=2) as tmp,
            tc.tile_pool(name="psum", bufs=8, space="PSUM") as psum,
        ):
            # level-1/2 pooled slabs, free = ch*(H*W) + h*W + w
            f1p = slab.tile([128, CH * H1 * W1], f32r)
            f2p = slab.tile([128, CH * H1 * W1], f32r)
            f1q = slab.tile([128, CH * H2 * W2], f32r)
            f2q = slab.tile([128, CH * H2 * W2], f32r)
            # level-1/2 Gram staging (natural layout G[w, h*W + w'])
            G1 = slab.tile([W1, H1 * W1], f16)
            G2 = slab.tile([W2, H2 * W2], f16)

            def sub_gram(l, g0, ngroups):
                """Matmuls+copies+store for L1/L2 rhs-groups starting at g0."""
                W, H, Grp = LEVELS[l]
                s1, s2 = (f1p, f2p) if l == 1 else (f1q, f2q)
                Gt = G1 if l == 1 else G2
                for g in range(g0, g0 + ngroups * Grp, Grp):
                    for t in range(Grp):
                        h = g + t
                        ps = psum.tile([128, 512], f32, tag="ps")
                        for ch in range(CH):
                            lhsT = s1[:, ch * H * W + h * W : ch * H * W + (h + 1) * W]
                            rhs = s2[:, ch * H * W + g * W : ch * H * W + (g + Grp) * W]
                            nc.tensor.matmul(
                                ps[:W, 0 : Grp * W], lhsT, rhs,
                                start=(ch == 0), stop=(ch == CH - 1),
                            )
                        nc.scalar.copy(
                            out=Gt[:, h * W : (h + 1) * W],
                            in_=ps[:W, t * W : (t + 1) * W],
                        )
                lo, hi = g0 * W, (g0 + ngroups * Grp) * W
                nc.sync.dma_start(out=gouts[l].ap()[:, lo:hi], in_=Gt[:, lo:hi])

            def emit_pool(h0, chunk, f1c, f2c):
                """Pooling + dependent L1/L2 gram work for one streamed chunk."""
                hc = chunk // 2
                qc = chunk // 4
                # ---- level0 -> level1 slab rows [h0/2, h0/2 + hc) ----
                t1s = []
                for idx, (t_sb, eng_h) in enumerate(((f1c, nc.vector), (f2c, nc.gpsimd))):
                    t1 = tmp.tile([128, CH * hc * W0], f32, tag=f"t1{idx}",
                                  name=f"t1{idx}")
                    a0 = bass.AP(t_sb.tensor, 0,
                                 [[t_sb.tensor.shape[1], 128],
                                  [chunk * W0, CH], [2 * W0, hc], [1, W0]])
                    a1 = bass.AP(t_sb.tensor, W0,
                                 [[t_sb.tensor.shape[1], 128],
                                  [chunk * W0, CH], [2 * W0, hc], [1, W0]])
                    d1 = bass.AP(t1.tensor, 0,
                                 [[t1.tensor.shape[1], 128],
                                  [hc * W0, CH], [W0, hc], [1, W0]])
                    eng_h.tensor_add(out=d1, in0=a0, in1=a1)
                    t1s.append(t1)
                for t1, t_slab in zip(t1s, (f1p, f2p)):
                    b0 = bass.AP(t1.tensor, 0,
                                 [[t1.tensor.shape[1], 128],
                                  [hc * W0, CH], [W0, hc], [2, W1]])
                    b1 = bass.AP(t1.tensor, 1,
                                 [[t1.tensor.shape[1], 128],
                                  [hc * W0, CH], [W0, hc], [2, W1]])
                    d2 = bass.AP(t_slab.tensor, (h0 // 2) * W1,
                                 [[t_slab.tensor.shape[1], 128],
                                  [H1 * W1, CH], [W1, hc], [1, W1]])
                    nc.vector.tensor_add(out=d2, in0=b0, in1=b1)

                # ---- level-1 Grams for rows just pooled ----
                sub_gram(1, h0 // 2, hc // LEVELS[1][2])

                # ---- level1 -> level2 slab rows [h0/4, h0/4 + qc) ----
                for t_slab, t_out in ((f1p, f1q), (f2p, f2q)):
                    eng = nc.vector if t_slab is f1p else nc.gpsimd
                    for ch in range(CH):
                        t2 = tmp.tile([128, qc * W1], f32, tag="t2", name="t2")
                        a0 = bass.AP(t_slab.tensor,
                                     ch * H1 * W1 + (h0 // 2) * W1,
                                     [[t_slab.tensor.shape[1], 128],
                                      [2 * W1, qc], [1, W1]])
                        a1 = bass.AP(t_slab.tensor,
                                     ch * H1 * W1 + (h0 // 2 + 1) * W1,
                                     [[t_slab.tensor.shape[1], 128],
                                      [2 * W1, qc], [1, W1]])
                        d1 = bass.AP(t2.tensor, 0,
                                     [[t2.tensor.shape[1], 128], [W1, qc], [1, W1]])
                        eng.tensor_add(out=d1, in0=a0, in1=a1)
                        b0 = bass.AP(t2.tensor, 0,
                                     [[t2.tensor.shape[1], 128], [W1, qc], [2, W2]])
                        b1 = bass.AP(t2.tensor, 1,
                                     [[t2.tensor.shape[1], 128], [W1, qc], [2, W2]])
                        d2 = bass.AP(t_out.tensor,
                                     ch * H2 * W2 + (h0 // 4) * W2,
                                     [[t_out.tensor.shape[1], 128],
                                      [W2, qc], [1, W2]])
                        nc.vector.tensor_add(out=d2, in0=b0, in1=b1)

                # ---- level-2 Grams whenever a full rhs-group (8 rows) ready ----
                Grp2 = LEVELS[2][2]
                prev_done = h0 // 4
                q_done = prev_done + qc
                for gidx in range(prev_done // Grp2, q_done // Grp2):
                    sub_gram(2, gidx * Grp2, 1)

            # ---------------- streamed pipeline ----------------
            pending_pool = None
            h0 = 0
            for ci, chunk in enumerate(chunk_list):
                f1c = stream.tile([128, CH * chunk * W0], f32r, tag="f1c", name="f1c")
                f2c = stream.tile([128, CH * chunk * W0], f32r, tag="f2c", name="f2c")
                for t_dram, t_sb, dma_eng in ((f1, f1c, nc.sync), (f2, f2c, nc.scalar)):
                    src = t_dram.ap()[:, :, h0 : h0 + chunk, :].transpose([1, 0, 2, 3])
                    dst = t_sb[:].rearrange("p (a b) -> p a b", a=CH)
                    dma_eng.dma_start(out=dst, in_=src)

                gc = gcp.tile([128, chunk * W0], f16, tag="gc", name="gc")
                # Gram matmuls: row pairs (ha, ha+1); one PSUM bank per pair
                for hp in range(chunk // 2):
                    ha = 2 * hp
                    ps = psum.tile([128, 512], f32, tag="ps", name="ps")
                    for t in range(2):
                        for ch in range(CH):
                            lhsT = f1c[:, ch * chunk * W0 + (ha + t) * W0:
                                       ch * chunk * W0 + (ha + t + 1) * W0]
                            rhs = f2c[:, ch * chunk * W0 + ha * W0:
                                      ch * chunk * W0 + (ha + 2) * W0]
                            nc.tensor.matmul(
                                ps[:, t * 256 : (t + 1) * 256], lhsT, rhs,
                                start=(ch == 0), stop=(ch == CH - 1),
                            )
                    # useful blocks: row ha -> ps[:,0:128]; row ha+1 -> ps[:,384:512]
                    # iteration (w, t, w'); contiguous dest gc[w, (ha+t)*128 + w']
                    src = bass.AP(
                        ps.tensor, 0, [[ps.tensor.shape[1], 128], [384, 2], [1, 128]]
                    )
                    dst = gc[:, ha * W0 : (ha + 2) * W0]
                    nc.vector.tensor_copy(out=dst, in_=src)
                half = (chunk // 2) * W0
                nc.sync.dma_start(
                    out=gouts[0].ap()[:, h0 * W0 : h0 * W0 + half],
                    in_=gc[:, 0:half],
                )
                nc.sync.dma_start(
                    out=gouts[0].ap()[:, h0 * W0 + half : (h0 + chunk) * W0],
                    in_=gc[:, half : chunk * W0],
                )

                # emit previous chunk's pooling AFTER this chunk's copies so
                # the scheduler prioritizes PSUM recycling
                if pending_pool is not None:
                    pending_pool()
                if ci == len(chunk_list) - 1:
                    # last chunk: emit its pooling chain immediately so the
                    # dependent L1/L2 tail starts as early as possible
                    emit_pool(h0, chunk, f1c, f2c)
                    pending_pool = None
                else:
                    pending_pool = (lambda h0=h0, chunk=chunk, f1c=f1c, f2c=f2c:
                                    emit_pool(h0, chunk, f1c, f2c))
                h0 += chunk
            if pending_pool is not None:
                pending_pool()

    nc.compile()
    return nc


def _get_nc():
    if "nc" not in _cached:
        _cached["nc"] = _build()
    return _cached["nc"]


def _extract(gfull, W, H, scale):
    """gfull [B, W, H*W] -> corr [B, 9, H, W]; corr[b,d,h,w] = g[b,w,h,w+d-4]*scale."""
    g = gfull.reshape(B, W, H, W)
    out = np.zeros((B, ND, H, W), np.float32)
    for d in range(ND):
        dp = d - MAX_DISP
        lo = max(0, -dp)
        hi = min(W, W - dp)
        wv = np.arange(lo, hi)
        sl = g[:, wv, :, wv + dp].astype(np.float32)  # [len(wv), B, H]
        out[:, d, :, lo:hi] = np.moveaxis(sl, 0, -1) * np.float32(scale)
    return out


def kernel(f1: np.ndarray, f2: np.ndarray):
    from concourse.bass_utils import run_bass_kernel_spmd

    nc = _get_nc()
    f1 = np.ascontiguousarray(np.asarray(f1, dtype=np.float32))
    f2 = np.ascontiguousarray(np.asarray(f2, dtype=np.float32))
    in_maps = [
        {
            "f1": f1[b].reshape(CH, 128, H0, W0),
            "f2": f2[b].reshape(CH, 128, H0, W0),
        }
        for b in range(B)
    ]
    res = run_bass_kernel_spmd(
        nc, in_maps, core_ids=list(range(B)), trace=_cached.get("trace", False)
    )
    _cached["last_result"] = res

    outs = []
    for l, (W, H, _) in enumerate(LEVELS):
        gfull = np.stack([res.results[b][f"g{l}"] for b in range(B)], axis=0)
        outs.append(_extract(gfull, W, H, SCALES[l]))
    return tuple(outs)
